# revision 4
# baseline (speedup 1.0000x reference)
"""Block-diagonal linear layer on 8 trn2 NeuronCores.

Reference op:  out = x @ tanh(W * mask).T
  x    [8192, 4096] f32
  W    [4096, 4096] f32, random inside 8 diagonal 512x512 blocks, 0 outside
  mask [4096, 4096] bool, True exactly on the 8 diagonal 512x512 blocks

tanh(0) == 0, so eff = tanh(W*mask) is block-diagonal: out[:, blk_k] depends
only on x[:, blk_k] and W[blk_k, blk_k].  Sharding: block k -> core k
(expert-style), zero inter-core communication.

Per-core device program (SPMD, same NEFF on all 8 cores).  All inputs are
host-pre-tiled so every DMA is contiguous per SBUF partition (this matters:
cold-start DMA is descriptor-latency-bound, ~200GB/s for the first ~5us,
so the startup-critical transfers must be few and exactly-needed bytes):

  wt3  [128,4,4,128] f16   wt3[p,o,c,j] = tanh(W)[128o+j, 128c+p]
                           (4 o-strips of eff^T, loaded individually)
  xp0a [128,4,128]   f16   first 128 batch cols of x_blk^T, i-chunked
  xp0b [128,4,128]   f16   next 128 batch cols
  xp0c [128,4,256]   f16   next 256 batch cols
  xt2  [128,16,4,512] f16  xt2[p,t,c,b] = x[512t+b, blk+128c+p]
                           (pieces t>=1 / quads for the steady state)
  ot   [512, 8192]   f16   out[:, blk].T

f16 keeps the PE at 1 col/cycle (the trn2 16-bit roofline; fp8 DoubleRow
would be 2x but its ~3.7e-2 quantization error fails the 2e-2 gate); K=512
dots give ~4e-4 relative error.

Schedule (from trace analysis):
  head   framework preamble ends ~7.2us.  The sync ring loads, in need
         order: eff strip0, xp0a, strip1, xp0b, strip2, strip3, xp0c —
         the first matmul group (o=0, 128 cols) needs only strip0+xp0a
         (256KB), landing ~9.9us.  The scalar ring loads pieces h1-3 and
         the quads.  6 warmup matmuls bridge 8.0->10.5us so PE activity is
         continuous and the HAM clock-gate opens (2.4GHz) at ~11.4us.
  body   matmuls run back-to-back at the warm roofline (~216ns/512 cols).
         PSUM->SBUF drains on vector only; scalar is a pure 2nd DMA ring.
  tail   the last quad is h-major; the final (o,h) group drains as two
         256-col copy+store halves on both rings, so the last 64KB store
         issues ~1us after the last matmul.
"""

from contextlib import ExitStack

import numpy as np

BLOCK = 512
NBLOCKS = 8
BATCH = 8192
N = BLOCK * NBLOCKS

KI = BLOCK // 128  # 4 contraction chunks of 128 (SBUF partition dim)
OT = BLOCK // 128  # 4 output-row tiles of 128
BT = 512           # batch tile (one PSUM bank of f32)
NB = BATCH // BT   # 16 batch tiles

LADDER = (128, 128, 256)  # first-piece batch sub-widths

_CACHED = {}


def _build_program():
    import concourse.bacc as bacc
    import concourse.bass as bass
    import concourse.mybir as mybir
    import concourse.tile as tile

    f16 = mybir.dt.float16
    f32 = mybir.dt.float32

    nc = bacc.Bacc(
        "TRN2",
        target_bir_lowering=False,
        debug=False,
        enable_asserts=False,
        num_devices=NBLOCKS,
    )

    wt3 = nc.dram_tensor("wt3", [128, OT, KI, 128], f16, kind="ExternalInput").ap()
    xp0 = [
        nc.dram_tensor(f"xp0{t}", [128, KI, w], f16, kind="ExternalInput").ap()
        for t, w in zip("abc", LADDER)
    ]
    xt2 = nc.dram_tensor("xt2", [128, NB, KI, BT], f16, kind="ExternalInput").ap()
    ot = nc.dram_tensor("ot", [BLOCK, BATCH], f16, kind="ExternalOutput").ap()

    QUAD = 2048              # batch columns per steady-state x-load DMA
    NQ = BATCH // QUAD       # 4 quads
    HT = QUAD // BT          # 4 batch tiles per quad

    with tile.TileContext(nc) as tc, ExitStack() as ctx:
        wpool = ctx.enter_context(tc.tile_pool(name="w", bufs=1))
        xpool = ctx.enter_context(tc.tile_pool(name="x", bufs=4))
        opool = ctx.enter_context(tc.tile_pool(name="o", bufs=2))
        pspool = ctx.enter_context(tc.tile_pool(name="ps", bufs=2, space="PSUM"))

        # PE warmup: 6 x 512-col matmuls (~2.6us cold) bridge the gap between
        # preamble end (~7.2us) and the first inputs landing (~10us), keeping
        # PE activity continuous so the HAM clock-gate's 3.4us busy window
        # completes early in the real stream.
        xwarm = wpool.tile([128, BT], f16, tag="warm", name="xwarm")
        nc.vector.memset(xwarm[:], 0.0)
        pw = pspool.tile([128, BT], f32, tag="pb0", name="warm")
        for r in range(6):
            nc.tensor.matmul(pw[:], xwarm[:, :128], xwarm[:], start=True, stop=True)

        # eff o-strips + first-piece ladder on the sync ring, interleaved in
        # consumption order.  eff2[:, o, i, :] is the stationary operand for
        # (o, i); each strip/ladder DMA is contiguous per partition.
        eff2 = wpool.tile([128, OT, KI, 128], f16, tag="e", name="eff2")
        xls = [
            wpool.tile([128, KI, w], f16, tag=f"xl{t}", name=f"xl{t}")
            for t, w in zip("abc", LADDER)
        ]

        nc.sync.dma_start(eff2[:, 0, :, :], wt3[:, 0, :, :])
        nc.sync.dma_start(xls[0][:], xp0[0][:])
        nc.sync.dma_start(eff2[:, 1, :, :], wt3[:, 1, :, :])
        nc.sync.dma_start(xls[1][:], xp0[1][:])
        nc.sync.dma_start(eff2[:, 2, :, :], wt3[:, 2, :, :])
        nc.sync.dma_start(eff2[:, 3, :, :], wt3[:, 3, :, :])
        nc.sync.dma_start(xls[2][:], xp0[2][:])

        # pieces h=1..3 of quad 0 on the scalar ring (sync is busy with the
        # critical set); quads 1-3 follow there too.
        xq0 = xpool.tile([128, HT - 1, KI, BT], f16, tag="x", name="xq0")
        for h in range(1, HT):
            nc.scalar.dma_start(xq0[:, h - 1, :, :], xt2[:, h, :, :])

        stgs0 = [
            opool.tile([128, QUAD], f16, tag=f"so{o}", name=f"st{o}_0")
            for o in range(OT)
        ]

        # first-piece ladder: one accumulation group per (sub-piece, o)
        off = 0
        for t, w in enumerate(LADDER):
            for o in range(OT):
                ps = pspool.tile([128, BT], f32, tag=f"pb{o}", name=f"psL{t}_{o}")
                for i in range(KI):
                    nc.tensor.matmul(
                        ps[:, 0:w],
                        eff2[:, o, i, :],
                        xls[t][:, i, :],
                        start=(i == 0),
                        stop=(i == KI - 1),
                    )
                nc.vector.tensor_copy(stgs0[o][:, off : off + w], ps[:, 0:w])
            off += w

        for q in range(NQ):
            if q == 0:
                xq = xq0
            else:
                xq = xpool.tile([128, HT, KI, BT], f16, tag="x", name=f"xq{q}")
                nc.scalar.dma_start(xq[:], xt2[:, HT * q : HT * (q + 1), :, :])

            if q == 0:
                # h-major over the remaining pieces of quad 0
                for h in range(1, HT):
                    for o in range(OT):
                        ps = pspool.tile(
                            [128, BT], f32, tag=f"pb{o}", name=f"ps{o}_0_{h}"
                        )
                        for i in range(KI):
                            nc.tensor.matmul(
                                ps[:],
                                eff2[:, o, i, :],
                                xq[:, h - 1, i, :],
                                start=(i == 0),
                                stop=(i == KI - 1),
                            )
                        nc.vector.tensor_copy(
                            stgs0[o][:, BT * h : BT * (h + 1)], ps[:]
                        )
                # per-o 512KB stores on the sync ring (its critical loads
                # have drained by now; scalar still streams x)
                for o in range(OT):
                    nc.sync.dma_start(
                        ot[128 * o : 128 * (o + 1), 0:QUAD], stgs0[o][:]
                    )
            elif q < NQ - 1:
                # weight-reuse order: explicit LDWEIGHTS per (o, i); the 4
                # matmuls that follow share the stationary operand,
                # accumulating into 4 interleaved h-banks
                for o in range(OT):
                    pss = [
                        pspool.tile(
                            [128, BT], f32, tag=f"pb{h}", name=f"ps{o}_{q}_{h}"
                        )
                        for h in range(HT)
                    ]
                    for i in range(KI):
                        nc.tensor.ldweights(eff2[:, o, i, :])
                        for h in range(HT):
                            nc.tensor.matmul(
                                pss[h][:],
                                eff2[:, o, i, :],
                                xq[:, h, i, :],
                                start=(i == 0),
                                stop=(i == KI - 1),
                            )
                    stg = opool.tile([128, QUAD], f16, tag=f"so{o}", name=f"st{o}_{q}")
                    for h in range(HT):
                        nc.vector.tensor_copy(stg[:, BT * h : BT * (h + 1)], pss[h][:])
                    # q1 stores on sync (scalar still streams loads); q2
                    # alternates so neither ring spins down
                    if q == 1 or o % 2 == 0:
                        eng = nc.sync
                    else:
                        eng = nc.scalar
                    eng.dma_start(
                        ot[128 * o : 128 * (o + 1), QUAD * q : QUAD * (q + 1)],
                        stg[:],
                    )
            else:
                # last quad, h-major with per-(o,h) copy + 128KB store; the
                # final group drains as two 256-col halves on both rings
                stgs = [
                    opool.tile([128, QUAD], f16, tag=f"so{o}", name=f"st{o}_{q}")
                    for o in range(OT)
                ]
                for h in range(HT):
                    for o in range(OT):
                        ps = pspool.tile(
                            [128, BT], f32, tag=f"pb{o}", name=f"ps{o}_{q}_{h}"
                        )
                        for i in range(KI):
                            nc.tensor.matmul(
                                ps[:],
                                eff2[:, o, i, :],
                                xq[:, h, i, :],
                                start=(i == 0),
                                stop=(i == KI - 1),
                            )
                        col0 = QUAD * q + BT * h
                        last = h == HT - 1 and o == OT - 1
                        if last:
                            for half, eng in enumerate((nc.sync, nc.scalar)):
                                sl = slice(BT * h + 256 * half, BT * h + 256 * (half + 1))
                                nc.vector.tensor_copy(
                                    stgs[o][:, sl], ps[:, 256 * half : 256 * (half + 1)]
                                )
                                eng.dma_start(
                                    ot[
                                        128 * o : 128 * (o + 1),
                                        col0 + 256 * half : col0 + 256 * (half + 1),
                                    ],
                                    stgs[o][:, sl],
                                )
                        else:
                            nc.vector.tensor_copy(
                                stgs[o][:, BT * h : BT * (h + 1)], ps[:]
                            )
                            eng = nc.sync if (h * OT + o) % 2 == 0 else nc.scalar
                            eng.dma_start(
                                ot[128 * o : 128 * (o + 1), col0 : col0 + BT],
                                stgs[o][:, BT * h : BT * (h + 1)],
                            )

    nc.compile()
    return nc


def get_program():
    if "nc" not in _CACHED:
        _CACHED["nc"] = _build_program()
    return _CACHED["nc"]


def make_in_maps(x: np.ndarray, W: np.ndarray):
    x = np.asarray(x, dtype=np.float32)
    W = np.asarray(W, dtype=np.float32)
    in_maps = []
    for k in range(NBLOCKS):
        sl = slice(BLOCK * k, BLOCK * (k + 1))
        xb = x[:, sl].astype(np.float16)  # [8192, 512]
        # xt2[p, t, c, b] = xb[512t + b, 128c + p]
        xt2 = np.ascontiguousarray(
            xb.reshape(NB, BT, KI, 128).transpose(3, 0, 2, 1)
        )
        E = np.tanh(W[sl, sl]).astype(np.float16)  # [512 o, 512 i]
        # wt3[p, o, c, j] = E[128o + j, 128c + p]
        wt3 = np.ascontiguousarray(
            E.reshape(OT, 128, KI, 128).transpose(3, 0, 2, 1)
        )
        m = {"xt2": xt2, "wt3": wt3}
        off = 0
        for t, w in zip("abc", LADDER):
            # xp0{t}[p, c, b] = xb[off + b, 128c + p]
            m[f"xp0{t}"] = np.ascontiguousarray(
                xb[off : off + w].reshape(w, KI, 128).transpose(2, 1, 0)
            )
            off += w
        in_maps.append(m)
    return in_maps


def assemble_output(results) -> np.ndarray:
    out = np.empty((BATCH, N), np.float32)
    for k in range(NBLOCKS):
        out[:, BLOCK * k : BLOCK * (k + 1)] = results[k]["ot"].T.astype(np.float32)
    return out


def kernel(x: np.ndarray, W: np.ndarray, mask: np.ndarray) -> np.ndarray:
    # mask is exactly the block-diagonal pattern (all-True inside each
    # diagonal 512 block); W is already zero off-block, so tanh(W*mask)
    # restricted to block k is tanh(W[blk_k, blk_k]).
    from concourse.bass_utils import run_bass_kernel_spmd

    nc = get_program()
    in_maps = make_in_maps(x, W)
    res = run_bass_kernel_spmd(nc, in_maps, list(range(NBLOCKS)))
    return assemble_output(res.results)


# revision 5
# speedup vs baseline: 1.0649x; 1.0649x over previous
"""Block-diagonal linear layer on 8 trn2 NeuronCores.

Reference op:  out = x @ tanh(W * mask).T
  x    [8192, 4096] f32
  W    [4096, 4096] f32, random inside 8 diagonal 512x512 blocks, 0 outside
  mask [4096, 4096] bool, True exactly on the 8 diagonal 512x512 blocks

tanh(0) == 0, so eff = tanh(W*mask) is block-diagonal: out[:, blk_k] depends
only on x[:, blk_k] and W[blk_k, blk_k].  Sharding: block k -> core k
(expert-style), zero inter-core communication.

Per-core device program (SPMD, same NEFF on all 8 cores).  Inputs are
host-pre-tiled so every DMA is contiguous per SBUF partition with >=4KB
descriptors — cold-start DMA is descriptor-rate-bound (~4x slower at 1KB
descriptors), so the startup-critical transfers must be few and big:

  xt2  [128,16,4,512] f16  xt2[p,t,c,b] = x[512t+b, blk+128c+p]
  wt2  [128, 4, 512]  f16  wt2[p,c,o]   = tanh(W)[blk+o, blk+128c+p]
  ot   [512, 8192]    f16  = out[:, blk].T

f16 keeps the PE at 1 col/cycle (the trn2 16-bit roofline; fp8 DoubleRow
would be 2x but its ~3.7e-2 quantization error fails the 2e-2 gate); K=512
dots give ~4e-4 relative error.

Schedule (from trace analysis):
  head   framework preamble ends ~7.2us.  The sync HWDGE ring streams all
         of x (4 pieces then 3 quads, 4-16KB descriptors; measured clean
         with no mid-stream underruns); eff goes on the gpsimd SWDGE ring
         as a third parallel lane.  The 1MB critical set (eff + piece h0)
         lands ~11.3us.  17 x 256-col warmup matmuls keep the PE busy
         8.0->11.6us so the HAM clock-gate opens (2.4GHz) with no idle gap
         and the real stream starts warm.
  body   matmuls run back-to-back at the warm roofline (~216ns/512 cols).
         PSUM->SBUF drains on vector only; scalar is a pure store ring.
  tail   the last quad is h-major with a copy + 128KB store per (o,h)
         group; the final group drains as two 256-col copy+store halves on
         both HWDGE rings, so the last 64KB store issues right after the
         last matmul.
"""

from contextlib import ExitStack

import numpy as np

BLOCK = 512
NBLOCKS = 8
BATCH = 8192
N = BLOCK * NBLOCKS

KI = BLOCK // 128  # 4 contraction chunks of 128 (SBUF partition dim)
OT = BLOCK // 128  # 4 output-row tiles of 128
BT = 512           # batch tile (one PSUM bank of f32)
NB = BATCH // BT   # 16 batch tiles

_CACHED = {}


def _build_program():
    import concourse.bacc as bacc
    import concourse.bass as bass
    import concourse.mybir as mybir
    import concourse.tile as tile

    f16 = mybir.dt.float16
    f32 = mybir.dt.float32

    nc = bacc.Bacc(
        "TRN2",
        target_bir_lowering=False,
        debug=False,
        enable_asserts=False,
        num_devices=NBLOCKS,
    )

    xt2 = nc.dram_tensor("xt2", [128, NB, KI, BT], f16, kind="ExternalInput").ap()
    wt2 = nc.dram_tensor("wt2", [128, KI, BLOCK], f16, kind="ExternalInput").ap()
    ot = nc.dram_tensor("ot", [BLOCK, BATCH], f16, kind="ExternalOutput").ap()

    QUAD = 2048              # batch columns per steady-state x-load DMA
    NQ = BATCH // QUAD       # 4 quads
    HT = QUAD // BT          # 4 batch tiles per quad

    with tile.TileContext(nc) as tc, ExitStack() as ctx:
        wpool = ctx.enter_context(tc.tile_pool(name="w", bufs=1))
        xpool = ctx.enter_context(tc.tile_pool(name="x", bufs=4))
        opool = ctx.enter_context(tc.tile_pool(name="o", bufs=2))
        pspool = ctx.enter_context(tc.tile_pool(name="ps", bufs=2, space="PSUM"))

        # PE warmup: 17 x 256-col matmuls (~3.6us cold) keep the PE busy from
        # preamble end (~8.0us) until the first inputs land (~11.3us), so the
        # HAM clock-gate's 3.4us busy window completes with no idle gap and
        # the real stream runs at 2.4GHz.  256-col granularity bounds how
        # long a leftover warmup can delay the first real matmul.
        xwarm = wpool.tile([128, BT], f16, tag="warm", name="xwarm")
        nc.vector.memset(xwarm[:], 0.0)
        pw = pspool.tile([128, BT], f32, tag="pb0", name="warm")
        for r in range(17):
            nc.tensor.matmul(
                pw[:, 0:256], xwarm[:, :128], xwarm[:, 0:256], start=True, stop=True
            )

        # eff on the gpsimd SWDGE ring — a third DMA lane that spins up in
        # parallel with the sync HWDGE ring carrying x.  One DMA, one
        # completion sem (also keeps hoisted LDWEIGHTS from head-of-line
        # blocking the PE queue on a partially-landed weight chunk).
        eff = wpool.tile([128, KI, BLOCK], f16, tag="e", name="eff")
        nc.gpsimd.dma_start(eff[:], wt2[:])

        # all of x on the sync ring in consumption order
        xq0 = xpool.tile([128, HT, KI, BT], f16, tag="x", name="xq0")
        for h in range(HT):
            nc.sync.dma_start(xq0[:, h, :, :], xt2[:, h, :, :])

        for q in range(NQ):
            if q == 0:
                xq = xq0
            else:
                xq = xpool.tile([128, HT, KI, BT], f16, tag="x", name=f"xq{q}")
                nc.sync.dma_start(xq[:], xt2[:, HT * q : HT * (q + 1), :, :])

            if q == 0:
                # h-major: one accumulation group per (h, o), copied as soon
                # as it completes, consuming the arriving pieces in order
                stgs = [
                    opool.tile([128, QUAD], f16, tag=f"so{o}", name=f"st{o}_0")
                    for o in range(OT)
                ]
                for h in range(HT):
                    for o in range(OT):
                        ps = pspool.tile(
                            [128, BT], f32, tag=f"pb{o}", name=f"ps{o}_0_{h}"
                        )
                        for i in range(KI):
                            nc.tensor.matmul(
                                ps[:],
                                eff[:, i, 128 * o : 128 * (o + 1)],
                                xq[:, h, i, :],
                                start=(i == 0),
                                stop=(i == KI - 1),
                            )
                        nc.vector.tensor_copy(
                            stgs[o][:, BT * h : BT * (h + 1)], ps[:]
                        )
                # per-o 512KB stores on the scalar ring (sync still owns
                # undrained x-load packets)
                for o in range(OT):
                    nc.scalar.dma_start(
                        ot[128 * o : 128 * (o + 1), 0:QUAD], stgs[o][:]
                    )
            elif q < NQ - 1:
                # weight-reuse order: explicit LDWEIGHTS per (o, i); the 4
                # matmuls that follow share the stationary operand,
                # accumulating into 4 interleaved h-banks
                for o in range(OT):
                    pss = [
                        pspool.tile(
                            [128, BT], f32, tag=f"pb{h}", name=f"ps{o}_{q}_{h}"
                        )
                        for h in range(HT)
                    ]
                    for i in range(KI):
                        nc.tensor.ldweights(eff[:, i, 128 * o : 128 * (o + 1)])
                        for h in range(HT):
                            nc.tensor.matmul(
                                pss[h][:],
                                eff[:, i, 128 * o : 128 * (o + 1)],
                                xq[:, h, i, :],
                                start=(i == 0),
                                stop=(i == KI - 1),
                            )
                    stg = opool.tile([128, QUAD], f16, tag=f"so{o}", name=f"st{o}_{q}")
                    for h in range(HT):
                        nc.vector.tensor_copy(stg[:, BT * h : BT * (h + 1)], pss[h][:])
                    # q1 stores on scalar (sync still streams loads); q2
                    # alternates so neither ring spins down
                    if q == 2 and o % 2 == 0:
                        eng = nc.sync
                    else:
                        eng = nc.scalar
                    eng.dma_start(
                        ot[128 * o : 128 * (o + 1), QUAD * q : QUAD * (q + 1)],
                        stg[:],
                    )
            else:
                # last quad, h-major with per-(o,h) copy + 128KB store; the
                # final group drains as two 256-col halves on both rings
                stgs = [
                    opool.tile([128, QUAD], f16, tag=f"so{o}", name=f"st{o}_{q}")
                    for o in range(OT)
                ]
                for h in range(HT):
                    for o in range(OT):
                        ps = pspool.tile(
                            [128, BT], f32, tag=f"pb{o}", name=f"ps{o}_{q}_{h}"
                        )
                        for i in range(KI):
                            nc.tensor.matmul(
                                ps[:],
                                eff[:, i, 128 * o : 128 * (o + 1)],
                                xq[:, h, i, :],
                                start=(i == 0),
                                stop=(i == KI - 1),
                            )
                        col0 = QUAD * q + BT * h
                        last = h == HT - 1 and o == OT - 1
                        if last:
                            for half, eng in enumerate((nc.sync, nc.scalar)):
                                sl = slice(BT * h + 256 * half, BT * h + 256 * (half + 1))
                                nc.vector.tensor_copy(
                                    stgs[o][:, sl], ps[:, 256 * half : 256 * (half + 1)]
                                )
                                eng.dma_start(
                                    ot[
                                        128 * o : 128 * (o + 1),
                                        col0 + 256 * half : col0 + 256 * (half + 1),
                                    ],
                                    stgs[o][:, sl],
                                )
                        else:
                            nc.vector.tensor_copy(
                                stgs[o][:, BT * h : BT * (h + 1)], ps[:]
                            )
                            eng = nc.sync if (h * OT + o) % 2 == 0 else nc.scalar
                            eng.dma_start(
                                ot[128 * o : 128 * (o + 1), col0 : col0 + BT],
                                stgs[o][:, BT * h : BT * (h + 1)],
                            )

    nc.compile()
    return nc


def get_program():
    if "nc" not in _CACHED:
        _CACHED["nc"] = _build_program()
    return _CACHED["nc"]


def make_in_maps(x: np.ndarray, W: np.ndarray):
    x = np.asarray(x, dtype=np.float32)
    W = np.asarray(W, dtype=np.float32)
    in_maps = []
    for k in range(NBLOCKS):
        sl = slice(BLOCK * k, BLOCK * (k + 1))
        xb = x[:, sl].astype(np.float16)  # [8192, 512]
        # xt2[p, t, c, b] = xb[512t + b, 128c + p]
        xt2 = np.ascontiguousarray(
            xb.reshape(NB, BT, KI, 128).transpose(3, 0, 2, 1)
        )
        E = np.tanh(W[sl, sl]).astype(np.float16)  # [512 o, 512 i]
        # wt2[p, c, o] = E[o, 128c + p]
        wt2 = np.ascontiguousarray(E.reshape(BLOCK, KI, 128).transpose(2, 1, 0))
        in_maps.append({"xt2": xt2, "wt2": wt2})
    return in_maps


def assemble_output(results) -> np.ndarray:
    out = np.empty((BATCH, N), np.float32)
    for k in range(NBLOCKS):
        out[:, BLOCK * k : BLOCK * (k + 1)] = results[k]["ot"].T.astype(np.float32)
    return out


def kernel(x: np.ndarray, W: np.ndarray, mask: np.ndarray) -> np.ndarray:
    # mask is exactly the block-diagonal pattern (all-True inside each
    # diagonal 512 block); W is already zero off-block, so tanh(W*mask)
    # restricted to block k is tanh(W[blk_k, blk_k]).
    from concourse.bass_utils import run_bass_kernel_spmd

    nc = get_program()
    in_maps = make_in_maps(x, W)
    res = run_bass_kernel_spmd(nc, in_maps, list(range(NBLOCKS)))
    return assemble_output(res.results)


# revision 6
# speedup vs baseline: 1.1124x; 1.0446x over previous
"""Block-diagonal linear layer on 8 trn2 NeuronCores.

Reference op:  out = x @ tanh(W * mask).T
  x    [8192, 4096] f32
  W    [4096, 4096] f32, random inside 8 diagonal 512x512 blocks, 0 outside
  mask [4096, 4096] bool, True exactly on the 8 diagonal 512x512 blocks

tanh(0) == 0, so eff = tanh(W*mask) is block-diagonal: out[:, blk_k] depends
only on x[:, blk_k] and W[blk_k, blk_k].  Sharding: block k -> core k
(expert-style), zero inter-core communication.

Per-core device program (SPMD, same NEFF on all 8 cores).  Inputs are
host-pre-tiled so every DMA is contiguous per SBUF partition with >=4KB
descriptors — cold-start DMA is descriptor-rate-bound (~4x slower at 1KB
descriptors), so the startup-critical transfers must be few and big:

  xt2  [128,16,4,512] f16  xt2[p,t,c,b] = x[512t+b, blk+128c+p]
  wt2  [128, 4, 512]  f16  wt2[p,c,o]   = tanh(W)[blk+o, blk+128c+p]
  ot   [512, 8192]    f16  = out[:, blk].T

f16 keeps the PE at 1 col/cycle (the trn2 16-bit roofline; fp8 DoubleRow
would be 2x but its ~3.7e-2 quantization error fails the 2e-2 gate); K=512
dots give ~4e-4 relative error.

Schedule (from trace analysis):
  head   framework preamble ends ~7.2us.  The sync HWDGE ring streams all
         of x (4 pieces then 3 quads, 4-16KB descriptors; measured clean
         with no mid-stream underruns); eff goes on the gpsimd SWDGE ring
         as a third parallel lane.  The 1MB critical set (eff + piece h0)
         lands ~11.3us.  17 x 256-col warmup matmuls keep the PE busy
         8.0->11.6us so the HAM clock-gate opens (2.4GHz) with no idle gap
         and the real stream starts warm.
  body   matmuls run back-to-back at the warm roofline (~216ns/512 cols).
         PSUM->SBUF drains on vector only; scalar is a pure store ring.
  tail   the last quad is h-major with a copy + 128KB store per (o,h)
         group; the final group drains as two 256-col copy+store halves on
         both HWDGE rings, so the last 64KB store issues right after the
         last matmul.
"""

from contextlib import ExitStack

import numpy as np

BLOCK = 512
NBLOCKS = 8
BATCH = 8192
N = BLOCK * NBLOCKS

KI = BLOCK // 128  # 4 contraction chunks of 128 (SBUF partition dim)
OT = BLOCK // 128  # 4 output-row tiles of 128
BT = 512           # batch tile (one PSUM bank of f32)
NB = BATCH // BT   # 16 batch tiles

_CACHED = {}


def _build_program():
    import concourse.bacc as bacc
    import concourse.bass as bass
    import concourse.mybir as mybir
    import concourse.tile as tile

    f16 = mybir.dt.float16
    f32 = mybir.dt.float32

    nc = bacc.Bacc(
        "TRN2",
        target_bir_lowering=False,
        debug=False,
        enable_asserts=False,
        num_devices=NBLOCKS,
    )

    xt2 = nc.dram_tensor("xt2", [128, NB, KI, BT], f16, kind="ExternalInput").ap()
    wt2 = nc.dram_tensor("wt2", [128, KI, BLOCK], f16, kind="ExternalInput").ap()
    ot = nc.dram_tensor("ot", [BLOCK, BATCH], f16, kind="ExternalOutput").ap()

    QUAD = 2048              # batch columns per steady-state x-load DMA
    NQ = BATCH // QUAD       # 4 quads
    HT = QUAD // BT          # 4 batch tiles per quad

    with tile.TileContext(nc) as tc, ExitStack() as ctx:
        wpool = ctx.enter_context(tc.tile_pool(name="w", bufs=1))
        xpool = ctx.enter_context(tc.tile_pool(name="x", bufs=4))
        opool = ctx.enter_context(tc.tile_pool(name="o", bufs=2))
        pspool = ctx.enter_context(tc.tile_pool(name="ps", bufs=2, space="PSUM"))

        # PE warmup: 20 x 256-col matmuls (~4.3us cold) keep the PE busy from
        # preamble end (~8.0us) until the first inputs land (~12us), so the
        # HAM clock-gate's 3.4us busy window completes with no idle gap and
        # the real stream runs at 2.4GHz.  256-col granularity bounds how
        # long a leftover warmup can delay the first real matmul.
        xwarm = wpool.tile([128, BT], f16, tag="warm", name="xwarm")
        nc.vector.memset(xwarm[:], 0.0)
        pw = pspool.tile([128, BT], f32, tag="pb0", name="warm")
        for r in range(20):
            nc.tensor.matmul(
                pw[:, 0:256], xwarm[:, :128], xwarm[:, 0:256], start=True, stop=True
            )

        # eff first on the sync ring (consistently the faster-starting
        # HWDGE ring); piece h0 alone on the scalar ring so the two 512KB
        # critical transfers stream in parallel.  eff lands as ONE sem,
        # which also keeps hoisted LDWEIGHTS from head-of-line blocking the
        # PE queue on a partially-landed weight chunk.
        eff = wpool.tile([128, KI, BLOCK], f16, tag="e", name="eff")
        nc.sync.dma_start(eff[:], wt2[:])

        xq0 = xpool.tile([128, HT, KI, BT], f16, tag="x", name="xq0")
        nc.scalar.dma_start(xq0[:, 0, :, :], xt2[:, 0, :, :])
        for h in range(1, HT):
            nc.sync.dma_start(xq0[:, h, :, :], xt2[:, h, :, :])

        for q in range(NQ):
            if q == 0:
                xq = xq0
            else:
                xq = xpool.tile([128, HT, KI, BT], f16, tag="x", name=f"xq{q}")
                nc.sync.dma_start(xq[:], xt2[:, HT * q : HT * (q + 1), :, :])

            if q == 0:
                # h-major: one accumulation group per (h, o), copied as soon
                # as it completes, consuming the arriving pieces in order
                stgs = [
                    opool.tile([128, QUAD], f16, tag=f"so{o}", name=f"st{o}_0")
                    for o in range(OT)
                ]
                for h in range(HT):
                    for o in range(OT):
                        ps = pspool.tile(
                            [128, BT], f32, tag=f"pb{o}", name=f"ps{o}_0_{h}"
                        )
                        for i in range(KI):
                            nc.tensor.matmul(
                                ps[:],
                                eff[:, i, 128 * o : 128 * (o + 1)],
                                xq[:, h, i, :],
                                start=(i == 0),
                                stop=(i == KI - 1),
                            )
                        nc.vector.tensor_copy(
                            stgs[o][:, BT * h : BT * (h + 1)], ps[:]
                        )
                # per-o 512KB stores on the scalar ring (sync still owns
                # undrained x-load packets)
                for o in range(OT):
                    nc.scalar.dma_start(
                        ot[128 * o : 128 * (o + 1), 0:QUAD], stgs[o][:]
                    )
            elif q < NQ - 1:
                # weight-reuse order: explicit LDWEIGHTS per (o, i); the 4
                # matmuls that follow share the stationary operand,
                # accumulating into 4 interleaved h-banks
                for o in range(OT):
                    pss = [
                        pspool.tile(
                            [128, BT], f32, tag=f"pb{h}", name=f"ps{o}_{q}_{h}"
                        )
                        for h in range(HT)
                    ]
                    for i in range(KI):
                        nc.tensor.ldweights(eff[:, i, 128 * o : 128 * (o + 1)])
                        for h in range(HT):
                            nc.tensor.matmul(
                                pss[h][:],
                                eff[:, i, 128 * o : 128 * (o + 1)],
                                xq[:, h, i, :],
                                start=(i == 0),
                                stop=(i == KI - 1),
                            )
                    stg = opool.tile([128, QUAD], f16, tag=f"so{o}", name=f"st{o}_{q}")
                    for h in range(HT):
                        nc.vector.tensor_copy(stg[:, BT * h : BT * (h + 1)], pss[h][:])
                    # q1 stores on scalar (sync still streams loads); q2
                    # alternates so neither ring spins down
                    if q == 2 and o % 2 == 0:
                        eng = nc.sync
                    else:
                        eng = nc.scalar
                    eng.dma_start(
                        ot[128 * o : 128 * (o + 1), QUAD * q : QUAD * (q + 1)],
                        stg[:],
                    )
            else:
                # last quad, h-major with per-(o,h) copy + 128KB store; the
                # final group drains as two 256-col halves on both rings
                stgs = [
                    opool.tile([128, QUAD], f16, tag=f"so{o}", name=f"st{o}_{q}")
                    for o in range(OT)
                ]
                for h in range(HT):
                    for o in range(OT):
                        ps = pspool.tile(
                            [128, BT], f32, tag=f"pb{o}", name=f"ps{o}_{q}_{h}"
                        )
                        for i in range(KI):
                            nc.tensor.matmul(
                                ps[:],
                                eff[:, i, 128 * o : 128 * (o + 1)],
                                xq[:, h, i, :],
                                start=(i == 0),
                                stop=(i == KI - 1),
                            )
                        col0 = QUAD * q + BT * h
                        last = h == HT - 1 and o == OT - 1
                        if last:
                            for half, eng in enumerate((nc.sync, nc.scalar)):
                                sl = slice(BT * h + 256 * half, BT * h + 256 * (half + 1))
                                nc.vector.tensor_copy(
                                    stgs[o][:, sl], ps[:, 256 * half : 256 * (half + 1)]
                                )
                                eng.dma_start(
                                    ot[
                                        128 * o : 128 * (o + 1),
                                        col0 + 256 * half : col0 + 256 * (half + 1),
                                    ],
                                    stgs[o][:, sl],
                                )
                        else:
                            nc.vector.tensor_copy(
                                stgs[o][:, BT * h : BT * (h + 1)], ps[:]
                            )
                            eng = nc.sync if (h * OT + o) % 2 == 0 else nc.scalar
                            eng.dma_start(
                                ot[128 * o : 128 * (o + 1), col0 : col0 + BT],
                                stgs[o][:, BT * h : BT * (h + 1)],
                            )

    nc.compile()
    return nc


def get_program():
    if "nc" not in _CACHED:
        _CACHED["nc"] = _build_program()
    return _CACHED["nc"]


def make_in_maps(x: np.ndarray, W: np.ndarray):
    x = np.asarray(x, dtype=np.float32)
    W = np.asarray(W, dtype=np.float32)
    in_maps = []
    for k in range(NBLOCKS):
        sl = slice(BLOCK * k, BLOCK * (k + 1))
        xb = x[:, sl].astype(np.float16)  # [8192, 512]
        # xt2[p, t, c, b] = xb[512t + b, 128c + p]
        xt2 = np.ascontiguousarray(
            xb.reshape(NB, BT, KI, 128).transpose(3, 0, 2, 1)
        )
        E = np.tanh(W[sl, sl]).astype(np.float16)  # [512 o, 512 i]
        # wt2[p, c, o] = E[o, 128c + p]
        wt2 = np.ascontiguousarray(E.reshape(BLOCK, KI, 128).transpose(2, 1, 0))
        in_maps.append({"xt2": xt2, "wt2": wt2})
    return in_maps


def assemble_output(results) -> np.ndarray:
    out = np.empty((BATCH, N), np.float32)
    for k in range(NBLOCKS):
        out[:, BLOCK * k : BLOCK * (k + 1)] = results[k]["ot"].T.astype(np.float32)
    return out


def kernel(x: np.ndarray, W: np.ndarray, mask: np.ndarray) -> np.ndarray:
    # mask is exactly the block-diagonal pattern (all-True inside each
    # diagonal 512 block); W is already zero off-block, so tanh(W*mask)
    # restricted to block k is tanh(W[blk_k, blk_k]).
    from concourse.bass_utils import run_bass_kernel_spmd

    nc = get_program()
    in_maps = make_in_maps(x, W)
    res = run_bass_kernel_spmd(nc, in_maps, list(range(NBLOCKS)))
    return assemble_output(res.results)
